# revision 1
# baseline (speedup 1.0000x reference)
"""Additive (Bahdanau) attention scoring kernel for Trainium2, 8-core SPMD.

Reference computation (B=16, S=4096, D=1024, all fp32):
    q      = target @ Wq.T                    # [B, D]
    k      = memory @ Wk.T                    # [B, S, D]
    scores = tanh(q[:, None, :] + k) @ v      # [B, S]
    out    = softmax(scores - 1e9 * mask, axis=-1)

Sharding: batch across the 8 cores (2 batches per core), weights replicated.

Host-side prep (layout only, no math): memory is transposed to [D, S] per
batch so the contraction dim lands on SBUF partitions, and its columns are
compacted to just the unmasked positions (padded with duplicates of the
first kept column to a 128-multiple, tail strip >= 256). Masked positions
contribute exactly 0 to the reference softmax (exp(-1e9) == 0 in fp32), so
skipping their k-matmul columns is algebraically exact.

Per-core device pipeline (python-unrolled, Tile-scheduled):
  - q^T via fp32r matmuls with target as the M=2 stationary and WqT as the
    N=512 moving operand (fp32r hard-faults the device for small moving N),
    transposed into per-partition bias layout through a DRAM bounce.
  - k^T tiles [e=128, s'=w] = WkT chunk.T @ memC chunk, fp32r accumulated
    over d. fp32r operands must be produced by a rounding compute op, so
    every DMA-landed operand gets a DVE cast into a separate f32r tile.
  - One ACT pass fuses the q-add and tanh (q as per-partition bias),
    writing f32r.
  - v-dot on the PE: psum[1, w] += v_chunk.T @ tanh_tile over the 8
    e-chunks; exp() applied in the ACT copy out of PSUM.
  - The exp strip is scattered back to full-S positions on device
    (DRAM bounce to [128, w/128], then indirect DMAs; duplicate pad
    indices are idempotent). scratch_full is zero-filled per batch, so
    masked positions are exactly 0.
  - Softmax finale per batch (no max-shift needed: |scores| <= sum|v| ~ 8,
    exp cannot overflow): [128, 32] esq load, mask multiply, free-dim
    reduce, ones-matmul partition reduce, reciprocal, per-partition scale.
"""

import os
from contextlib import ExitStack

import numpy as np

import concourse.tile as tile
from concourse import bacc, mybir
import concourse.bass as bass

B, S, D = 16, 4096, 1024
N_CORES = 8
NB = B // N_CORES  # batches per core
P = 128
DC = D // P        # contraction chunks
ET = D // P        # e tiles
SW = 512           # full strip width along compacted s
SQ = S // P        # 32: free dim of the [128, 32] softmax layout

F32 = mybir.dt.float32
F32R = mybir.dt.float32r
U32 = mybir.dt.uint32
AF = mybir.ActivationFunctionType

_CACHE = {}


def strip_widths(max_kept):
    """Strip widths covering max_kept compacted columns: full 512-wide strips
    plus a 128-granular tail of at least 256 (small moving-N fp32r matmuls
    hard-fault the device)."""
    total = max(512, ((max_kept + 127) // 128) * 128)
    widths = [SW] * (total // SW)
    rem = total % SW
    if rem:
        widths.append(max(256, rem))
    return tuple(widths)


def _build_program(stage, widths):
    """stage: 1 = dma+matmul+tanh only, 2 = +vdot/exp/scatter, 27 = full."""
    s_pad = sum(widths)
    nslot = s_pad // P  # indirect-scatter slots per batch

    nc = bacc.Bacc("TRN2", target_bir_lowering=False, debug=False)

    memC = nc.dram_tensor("memC", [NB, D, s_pad], F32, kind="ExternalInput").ap()
    wkT = nc.dram_tensor("wkT", [D, D], F32, kind="ExternalInput").ap()
    wqT = nc.dram_tensor("wqT", [D, D], F32, kind="ExternalInput").ap()
    tgtT = nc.dram_tensor("tgtT", [D, NB], F32, kind="ExternalInput").ap()
    vT = nc.dram_tensor("vT", [P, ET], F32, kind="ExternalInput").ap()
    keep = nc.dram_tensor("keep", [NB, P, SQ], F32, kind="ExternalInput").ap()
    idxs = nc.dram_tensor("idxs", [NB, nslot, P], U32, kind="ExternalInput").ap()
    out = nc.dram_tensor("out", [NB, P, SQ], F32, kind="ExternalOutput").ap()

    with tile.TileContext(nc) as tc, ExitStack() as ctx:
        consts = ctx.enter_context(tc.tile_pool(name="consts", bufs=1))
        mem_pool = ctx.enter_context(tc.tile_pool(name="mem", bufs=2))
        tt_pool = ctx.enter_context(tc.tile_pool(name="tt", bufs=4))
        strip_pool = ctx.enter_context(tc.tile_pool(name="strip", bufs=2))
        fin_pool = ctx.enter_context(tc.tile_pool(name="fin", bufs=2))
        kps_pool = ctx.enter_context(tc.tile_pool(name="kps", bufs=4, space="PSUM"))
        vd_pool = ctx.enter_context(tc.tile_pool(name="vd", bufs=2, space="PSUM"))
        sm_pool = ctx.enter_context(tc.tile_pool(name="smps", bufs=2, space="PSUM"))
        dram_pool = ctx.enter_context(tc.tile_pool(name="scratch", bufs=2, space="DRAM"))

        # --- small constants (cheap DMAs first) ---
        tgt_sb = consts.tile([P, DC * NB], F32)
        for dc in range(DC):
            nc.sync.dma_start(tgt_sb[:, dc * NB:(dc + 1) * NB], tgtT[dc * P:(dc + 1) * P, :])
        tgt_r = consts.tile([P, DC * NB], F32R)
        nc.vector.tensor_copy(tgt_r[:], tgt_sb[:])
        v_sb = consts.tile([P, ET], F32)
        nc.sync.dma_start(v_sb[:], vT[:, :])
        v_r = consts.tile([P, ET], F32R)
        nc.vector.tensor_copy(v_r[:], v_sb[:])
        keep_sb = consts.tile([P, NB * SQ], F32)
        for b in range(NB):
            nc.sync.dma_start(keep_sb[:, b * SQ:(b + 1) * SQ], keep[b])
        idx_sb = consts.tile([P, NB * nslot], U32)
        for b in range(NB):
            nc.sync.dma_start(
                idx_sb[:, b * nslot:(b + 1) * nslot],
                idxs[b].rearrange("slot p -> p slot"),
            )
        ones_sb = consts.tile([P, P], F32)
        nc.vector.memset(ones_sb[:], 1.0)
        zero_sb = consts.tile([P, (S + P) // P], F32)
        nc.vector.memset(zero_sb[:], 0.0)

        # --- weights: Wq first (the q matmuls below are first in PE order),
        # then Wk. The two f32 landing buffers share one pool slot (their
        # lifetimes are sequential) to stay inside SBUF.
        wq_r = consts.tile([P, DC * D], F32R)
        wq_sb = consts.tile([P, DC * D], F32, tag="wstage", name="wq_sb")
        for dc in range(DC):
            nc.sync.dma_start(wq_sb[:, dc * D:(dc + 1) * D], wqT[dc * P:(dc + 1) * P, :])
            nc.vector.tensor_copy(wq_r[:, dc * D:(dc + 1) * D], wq_sb[:, dc * D:(dc + 1) * D])
        wk_r = consts.tile([P, DC * D], F32R)
        wk_sb = consts.tile([P, DC * D], F32, tag="wstage", name="wk_sb")
        for dc in range(DC):
            nc.sync.dma_start(wk_sb[:, dc * D:(dc + 1) * D], wkT[dc * P:(dc + 1) * P, :])
            nc.vector.tensor_copy(wk_r[:, dc * D:(dc + 1) * D], wk_sb[:, dc * D:(dc + 1) * D])

        q_sb = consts.tile([P, NB * ET], F32)

        # q[b, e] = sum_d target[b, d] * Wq[e, d]: fp32r with target as the
        # M=2 stationary and WqT as the N=512 moving operand. The [2, 1024]
        # result is transposed into per-partition bias layout [128, 16]
        # (b-major columns) through a DRAM bounce.
        q_row = consts.tile([NB, D], F32)
        for j in range(D // SW):
            q_ps2 = sm_pool.tile([NB, SW], F32, tag="small", name="q_ps2")
            for dc in range(DC):
                nc.tensor.matmul(
                    q_ps2[:],
                    tgt_r[:, dc * NB:(dc + 1) * NB],
                    wq_r[:, dc * D + j * SW: dc * D + (j + 1) * SW],
                    start=(dc == 0),
                    stop=(dc == DC - 1),
                )
            nc.vector.tensor_copy(q_row[:, j * SW:(j + 1) * SW], q_ps2[:])
        qscr = dram_pool.tile([NB, D], F32, tag="qscr", name="qscr")
        nc.sync.dma_start(qscr[:], q_row[:])
        for b in range(NB):
            nc.sync.dma_start(
                q_sb[:, b * ET:(b + 1) * ET],
                qscr[b].rearrange("(et p) -> p et", p=P),
            )

        def emit_vd(vd_ps, tts, c, w):
            nc.tensor.matmul(
                vd_ps[:, :w],
                v_r[:, c:c + 1],
                tts[c][:, :w],
                start=(c == 0),
                stop=(c == ET - 1),
            )

        scrfs = []
        for b in range(NB):
            # exp strips land contiguously in compact scratch, each strip
            # scattered to its full-S positions right away (pads go to the
            # trash cell at S)
            scrf = dram_pool.tile([1, S + P], F32, tag="scrf", name="scrf")
            nc.sync.dma_start(scrf.rearrange("o (p f) -> (o p) f", p=P), zero_sb[:])
            scrfs.append(scrf)
            scratch_cb = dram_pool.tile([1, s_pad], F32, tag="scrc", name="scrc")
            off = 0
            for sp, w in enumerate(widths):
                mem_sb = mem_pool.tile([P, DC * SW], F32)
                mem_r = mem_pool.tile([P, DC * SW], F32R, tag="mem_r", name="mem_r")
                for dc in range(DC):
                    nc.sync.dma_start(
                        mem_sb[:, dc * SW:dc * SW + w],
                        memC[b, dc * P:(dc + 1) * P, off:off + w],
                    )
                    nc.vector.tensor_copy(
                        mem_r[:, dc * SW:dc * SW + w], mem_sb[:, dc * SW:dc * SW + w]
                    )
                vd_ps = vd_pool.tile([1, SW], F32, tag="vd", name="vd_ps")
                tts = []
                for et in range(ET):
                    k_ps = kps_pool.tile([P, SW], F32, tag="k", name="k_ps")
                    for dc in range(DC):
                        nc.tensor.matmul(
                            k_ps[:, :w],
                            wk_r[:, dc * D + et * P: dc * D + (et + 1) * P],
                            mem_r[:, dc * SW:dc * SW + w],
                            start=(dc == 0),
                            stop=(dc == DC - 1),
                        )
                    tt = tt_pool.tile([P, SW], F32R, tag="tt", name="tt")
                    nc.scalar.activation(
                        tt[:, :w], k_ps[:, :w], AF.Tanh,
                        bias=q_sb[:, b * ET + et: b * ET + et + 1],
                    )
                    tts.append(tt)
                    # keep the PE stream 2 e-tiles ahead of the v-dot so it
                    # never stalls waiting on the ACT tanh
                    if stage >= 2 and et >= 2:
                        emit_vd(vd_ps, tts, et - 2, w)
                if stage < 2:
                    if sp == len(widths) - 1:
                        dbg = fin_pool.tile([P, SQ], F32, tag="outt", name="dbg")
                        nc.vector.tensor_copy(dbg[:], tts[7][:, :SQ])
                        nc.sync.dma_start(out[b], dbg[:])
                    off += w
                    continue
                emit_vd(vd_ps, tts, ET - 2, w)
                emit_vd(vd_ps, tts, ET - 1, w)

                strip_sb = strip_pool.tile([1, SW], F32, tag="strip", name="strip_sb")
                nc.scalar.activation(strip_sb[:, :w], vd_ps[:, :w], AF.Exp)
                nc.sync.dma_start(scratch_cb[:, off:off + w], strip_sb[:, :w])
                # scatter this strip's exp values to their full-S positions.
                # HW consumes one offset per in_-contiguous descriptor run,
                # so arbitrary positions need [128, 1] single-element rows.
                f = w // P
                sc_sb = strip_pool.tile([P, SW // P], F32, tag="scsb", name="sc_sb", bufs=8)
                nc.sync.dma_start(
                    sc_sb[:, :f],
                    scratch_cb[:, off:off + w].rearrange("o (p f) -> (o p) f", f=f),
                )
                for jj in range(f):
                    col = b * nslot + (off // P) + jj
                    nc.gpsimd.indirect_dma_start(
                        out=scrf.rearrange("o (s w2) -> (o s) w2", w2=1),
                        out_offset=bass.IndirectOffsetOnAxis(
                            ap=idx_sb[:, col:col + 1], axis=0
                        ),
                        in_=sc_sb[:, jj:jj + 1],
                        in_offset=None,
                    )
                off += w

        # finales AFTER both batches' compute: the ones-matmuls are in PE
        # program order, so batch 0's finale must not sit between the two
        # batches' k-matmul streams (PE would stall on the scatter chain)
        for b in range(NB):
            if stage < 2:
                continue
            # --- masked softmax finale for batch b ---
            esq = fin_pool.tile([P, SQ], F32, tag="esq", name="esq")
            nc.sync.dma_start(
                esq[:], scrfs[b][:, :S].rearrange("o (p f) -> (o p) f", p=P)
            )
            if stage < 25:
                outt = fin_pool.tile([P, SQ], F32, tag="outt", name="outt")
                nc.vector.tensor_copy(outt[:], esq[:])
                nc.sync.dma_start(out[b], outt[:])
                continue
            em = fin_pool.tile([P, SQ], F32, tag="em", name="em")
            part = fin_pool.tile([P, 1], F32, tag="part", name="part")
            nc.vector.tensor_mul(em[:], esq[:], keep_sb[:, b * SQ:(b + 1) * SQ])
            nc.vector.reduce_sum(part[:], em[:], axis=mybir.AxisListType.X)
            if stage < 26:
                outt = fin_pool.tile([P, SQ], F32, tag="outt", name="outt")
                nc.vector.tensor_copy(outt[:], em[:])
                nc.sync.dma_start(out[b], outt[:])
                continue
            tot_ps = sm_pool.tile([P, 1], F32, tag="small", name="tot_ps")
            nc.tensor.matmul(tot_ps[:], ones_sb[:], part[:], start=True, stop=True)
            recip = fin_pool.tile([P, 1], F32, tag="recip", name="recip")
            nc.vector.reciprocal(recip[:], tot_ps[:])
            outt = fin_pool.tile([P, SQ], F32, tag="outt", name="outt")
            nc.vector.tensor_scalar_mul(outt[:], em[:], recip[:, 0:1])
            nc.sync.dma_start(out[b], outt[:])

    nc.compile()
    return nc


def get_program(stage=None, widths=None):
    if stage is None:
        stage = int(os.environ.get("KERNEL_STAGE", "27"))
    assert widths is not None
    key = (stage, widths)
    if key not in _CACHE:
        _CACHE[key] = _build_program(stage, widths)
    return _CACHE[key]


def prepare_in_maps(memory, target, memory_mask, Wq, Wk, v):
    memory = np.asarray(memory, dtype=np.float32)
    target = np.asarray(target, dtype=np.float32)
    Wq = np.asarray(Wq, dtype=np.float32)
    Wk = np.asarray(Wk, dtype=np.float32)
    v = np.asarray(v, dtype=np.float32)
    mask = np.asarray(memory_mask)

    # host-side sharding / layout prep (no arithmetic)
    keep_bool = ~mask                                                # [B, S]
    widths = strip_widths(int(keep_bool.sum(1).max()))
    s_pad = sum(widths)

    memT = memory.transpose(0, 2, 1)                                 # [B, D, S] view
    kept_pad = np.empty((B, s_pad), dtype=np.int64)
    scat_idx = np.empty((B, s_pad), dtype=np.int64)
    for b in range(B):
        k = np.flatnonzero(keep_bool[b])
        kept_pad[b, :len(k)] = k
        kept_pad[b, len(k):] = k[0]  # pad data: duplicate first kept column
        scat_idx[b, :len(k)] = k
        scat_idx[b, len(k):] = S     # pad scatter target: trash cell at S
    memC = np.empty((B, D, s_pad), dtype=np.float32)
    for b in range(B):
        memC[b] = memT[b][:, kept_pad[b]]

    # scatter offsets in per-strip slot order: strip of width w at compact
    # offset `off` bounces to SBUF [128, w/128] with element (p, jj) holding
    # compact position off + p*(w/128) + jj
    slot_list = []
    off = 0
    for w in widths:
        f = w // P
        block = scat_idx[:, off:off + w].reshape(B, P, f)
        for jj in range(f):
            slot_list.append(block[:, :, jj])
        off += w
    idxs = np.stack(slot_list, axis=1).astype(np.uint32)             # [B, nslot, P]

    wkT = np.ascontiguousarray(Wk.T)                                 # [D, D]
    wqT = np.ascontiguousarray(Wq.T)                                 # [D, D]
    tgtT = np.ascontiguousarray(target.T)                            # [D, B]
    vT = np.ascontiguousarray(v.reshape(ET, P).T)                    # [P, ET]
    keep = np.ascontiguousarray(
        keep_bool.astype(np.float32).reshape(B, P, SQ))              # [B, P, SQ]

    in_maps = [
        {
            "memC": np.ascontiguousarray(memC[c * NB:(c + 1) * NB]),
            "wkT": wkT,
            "wqT": wqT,
            "tgtT": np.ascontiguousarray(tgtT[:, c * NB:(c + 1) * NB]),
            "vT": vT,
            "keep": np.ascontiguousarray(keep[c * NB:(c + 1) * NB]),
            "idxs": np.ascontiguousarray(idxs[c * NB:(c + 1) * NB]),
        }
        for c in range(N_CORES)
    ]
    return in_maps, widths


def gather_output(results):
    out = np.empty((B, S), dtype=np.float32)
    for c in range(N_CORES):
        out[c * NB:(c + 1) * NB] = results[c]["out"].reshape(NB, S)
    return out


def kernel(memory, target, memory_mask, Wq, Wk, v):
    from concourse.bass_utils import run_bass_kernel_spmd

    in_maps, widths = prepare_in_maps(memory, target, memory_mask, Wq, Wk, v)
    nc = get_program(widths=widths)
    res = run_bass_kernel_spmd(nc, in_maps, list(range(N_CORES)))
    return gather_output(res.results)



# revision 24
# speedup vs baseline: 1.0843x; 1.0843x over previous
"""Additive (Bahdanau) attention scoring kernel for Trainium2, 8-core SPMD.

Reference computation (B=16, S=4096, D=1024, all fp32):
    q      = target @ Wq.T                    # [B, D]
    k      = memory @ Wk.T                    # [B, S, D]
    scores = tanh(q[:, None, :] + k) @ v      # [B, S]
    out    = softmax(scores - 1e9 * mask, axis=-1)

Sharding: batch across the 8 cores (2 batches per core), weights replicated.

Host-side prep (layout only, no arithmetic): masked positions contribute
exactly 0 to the reference softmax (exp(-1e9) == 0 in fp32), so memory is
compacted to the unmasked columns per batch (padded to a 128 multiple with
duplicates of the first kept column; pads are zeroed on device via padmask
before the softmax sum). Layout is strip-blocked [P, dc-major] so each strip
is ONE contiguous-per-partition DMA. The compact softmax result is
unscattered to full S on the host (inverse of the input gather).

Device design (v2 — s-on-partitions):
  - k^T tiles [s=128, e=512] = memchunk.T @ Wk chunk, accumulated over the
    8 d-chunks in PSUM. Stationary = mem chunk (bf16, FWL), moving = Wk
    (bf16). s lands on PSUM partitions, so the v-dot becomes a free-dim
    reduction on the DVE instead of PE matmuls.
  - ACT drains PSUM immediately (copy -> bf16 SBUF), which decouples the
    PE from the q-dependent scoring chain (PSUM banks never back up).
  - DVE: kq = kraw + q_bcast, ACT: tt = tanh(kq), DVE: score column =
    tensor_tensor_reduce(tt * v_bcast) chained across the two e-halves.
  - q = target @ Wq.T on PE (tiny, [2, 1024]), broadcast to all partitions
    via gpsimd partition_broadcast. q-matmuls are emitted a few s-blocks
    into the PE stream so the Wq DMA (second queue) has landed by then.
  - Softmax finale per batch on compact [128, NSQ] layout: exp (ACT),
    padmask-multiply + row-sum in one DVE ttr, ones-matmul partition
    reduce, reciprocal, scale. No max-shift needed: |scores| <= sum|v| ~ 8.
  - DMA: sync queue = strip0 + Wk(dc0-3) + batch-0 strips; scalar queue =
    consts + Wk(dc4-7) + Wq + batch-1 strips; casts f32->bf16 on gpsimd.
"""

import os
from contextlib import ExitStack

import numpy as np

import concourse.tile as tile
from concourse import bacc, mybir
import concourse.bass as bass

B, S, D = 16, 4096, 1024
N_CORES = 8
NB = B // N_CORES  # batches per core
P = 128
DC = D // P        # contraction chunks (8)
EH = 2             # e halves (2 x 512)
SW = 512           # max strip width along compacted s

F32 = mybir.dt.float32
BF16 = mybir.dt.bfloat16
AF = mybir.ActivationFunctionType
ALU = mybir.AluOpType

_CACHE = {}
_UNSCATTER = {}

Q_AT = 8  # emit the q matmuls after this many s-blocks of k-matmuls


def make_widths(max_kept):
    """Strip widths covering max_kept compacted columns (128-granular).
    Two small leading strips let the PE start before the big DMAs land."""
    total = max(256, ((max_kept + 127) // 128) * 128)
    ws = []
    rem = total
    for wt in (256, 256):
        if rem >= wt + 128:
            ws.append(wt)
            rem -= wt
    while rem > SW:
        ws.append(SW)
        rem -= SW
    if rem:
        ws.append(rem)
    return tuple(ws)


def _build_program(stage, widths):
    s_pad = sum(widths)
    NSQ = s_pad // P

    nc = bacc.Bacc("TRN2", target_bir_lowering=False, debug=False)

    memS = nc.dram_tensor("memS", [NB, P, DC * s_pad], F32, kind="ExternalInput").ap()
    wkT = nc.dram_tensor("wkT", [D, D], F32, kind="ExternalInput").ap()
    wqT = nc.dram_tensor("wqT", [D, D], F32, kind="ExternalInput").ap()
    tgtT = nc.dram_tensor("tgtT", [D, NB], F32, kind="ExternalInput").ap()
    vrep = nc.dram_tensor("vrep", [P, D], F32, kind="ExternalInput").ap()
    padm = nc.dram_tensor("padm", [NB, P, NSQ], F32, kind="ExternalInput").ap()
    selm = nc.dram_tensor("selm", [NB, NB * P], F32, kind="ExternalInput").ap()
    out = nc.dram_tensor("out", [NB, P, NSQ], F32, kind="ExternalOutput").ap()

    with tile.TileContext(nc) as tc, ExitStack() as ctx:
        consts = ctx.enter_context(tc.tile_pool(name="consts", bufs=1))
        wstage = ctx.enter_context(tc.tile_pool(name="wstage", bufs=4))
        memf_pool = ctx.enter_context(tc.tile_pool(name="memf", bufs=2))
        memb_pool = ctx.enter_context(tc.tile_pool(name="memb", bufs=3))
        kraw_pool = ctx.enter_context(tc.tile_pool(name="kraw", bufs=16))
        kq_pool = ctx.enter_context(tc.tile_pool(name="kq", bufs=4))
        tt_pool = ctx.enter_context(tc.tile_pool(name="tt", bufs=4))
        scr_pool = ctx.enter_context(tc.tile_pool(name="scr", bufs=2))
        s0_pool = ctx.enter_context(tc.tile_pool(name="s0", bufs=2))
        score_pool = ctx.enter_context(tc.tile_pool(name="score", bufs=2))
        fin_pool = ctx.enter_context(tc.tile_pool(name="fin", bufs=2))
        kps_pool = ctx.enter_context(tc.tile_pool(name="kps", bufs=6, space="PSUM"))
        sm_pool = ctx.enter_context(tc.tile_pool(name="smps", bufs=2, space="PSUM"))
        dram_pool = ctx.enter_context(tc.tile_pool(name="scratch", bufs=1, space="DRAM"))

        strips = []
        off = 0
        for w in widths:
            strips.append((off, w))
            off += w

        # ---- strip0 DMA first on the sync queue (gates PE start) ----
        mem_f0 = memf_pool.tile([P, DC * SW], F32, tag="memf", name="mem_f")
        w0 = widths[0]
        nc.sync.dma_start(mem_f0[:, :DC * w0], memS[0, :, :DC * w0])
        mem_t0 = memb_pool.tile([P, DC * SW], BF16, tag="memb", name="mem_t")
        CAST = nc.gpsimd if os.environ.get("KERNEL_CAST", "gpsimd") == "gpsimd" else nc.vector
        CAST.tensor_copy(mem_t0[:, :DC * w0], mem_f0[:, :DC * w0])

        # ---- small consts on the scalar queue ----
        tgt_sb = consts.tile([P, DC * NB], F32)
        for dc in range(DC):
            nc.scalar.dma_start(tgt_sb[:, dc * NB:(dc + 1) * NB], tgtT[dc * P:(dc + 1) * P, :])
        tgt_b = consts.tile([P, DC * NB], BF16)
        nc.vector.tensor_copy(tgt_b[:], tgt_sb[:])
        v_sb = consts.tile([P, D], F32)
        nc.scalar.dma_start(v_sb[:], vrep[:, :])
        v_b = consts.tile([P, D], BF16)
        nc.vector.tensor_copy(v_b[:], v_sb[:])
        pad_sb = consts.tile([P, NB * NSQ], F32)
        for b in range(NB):
            nc.scalar.dma_start(pad_sb[:, b * NSQ:(b + 1) * NSQ], padm[b])
        ones_sb = consts.tile([P, P], F32)
        nc.vector.memset(ones_sb[:], 1.0)
        # sel[k, b*P+m] = (k == b): K=2 stationary that selects batch b's q
        # row and broadcasts it across all 128 output partitions. Shipped
        # from the host (DVE memset cannot write at base partition 1).
        sel_f = consts.tile([NB, NB * P], F32)
        nc.scalar.dma_start(sel_f[:], selm[:, :])
        sel = consts.tile([NB, NB * P], BF16)
        nc.vector.tensor_copy(sel[:], sel_f[:])

        # ---- weights: Wk split across both queues (dc0-3 sync, dc4-7
        # scalar), chunk-rotating f32 landing tiles; Wq after Wk on scalar.
        wk_b = consts.tile([P, DC * D], BF16)
        for dc in range(DC):
            eng = nc.sync if dc < 4 else nc.scalar
            for eh in range(EH):
                wkc = wstage.tile([P, SW], F32, tag="wkc", name="wkc")
                eng.dma_start(wkc[:], wkT[dc * P:(dc + 1) * P, eh * SW:(eh + 1) * SW])
                nc.vector.tensor_copy(wk_b[:, dc * D + eh * SW: dc * D + (eh + 1) * SW], wkc[:])
        wq_b = consts.tile([P, DC * D], BF16)
        for dc in range(DC):
            wqc = wstage.tile([P, D], F32, tag="wqc", name="wqc")
            nc.scalar.dma_start(wqc[:], wqT[dc * P:(dc + 1) * P, :])
            nc.vector.tensor_copy(wq_b[:, dc * D:(dc + 1) * D], wqc[:])

        q_row = consts.tile([NB, D], BF16)
        q_bc = consts.tile([P, NB * D], F32)

        def emit_q():
            # q[b, e] = sum_d target[b, d] * Wq[e, d]; result row layout
            # [2, 1024] bf16, then partition-broadcast per batch via a K=1
            # ones outer product on the PE (out[m, n] = q[n] for all m).
            for j in range(EH):
                q_ps = sm_pool.tile([NB, SW], F32, tag="small", name="q_ps")
                for dc in range(DC):
                    nc.tensor.matmul(
                        q_ps[:],
                        tgt_b[:, dc * NB:(dc + 1) * NB],
                        wq_b[:, dc * D + j * SW: dc * D + (j + 1) * SW],
                        start=(dc == 0),
                        stop=(dc == DC - 1),
                    )
                nc.vector.tensor_copy(q_row[:, j * SW:(j + 1) * SW], q_ps[:])
            for b in range(NB):
                for j in range(EH):
                    qb_ps = sm_pool.tile([P, SW], F32, tag="small", name="qb_ps")
                    nc.tensor.matmul(
                        qb_ps[:],
                        sel[:, b * P:(b + 1) * P],
                        q_row[:, j * SW:(j + 1) * SW],
                        start=True,
                        stop=True,
                    )
                    nc.vector.tensor_copy(
                        q_bc[:, b * D + j * SW: b * D + (j + 1) * SW], qb_ps[:]
                    )

        def emit_score(b, jg, kraws, score_sb):
            s0t = s0_pool.tile([P, EH], F32, tag="s0", name="s0t")
            for eh in range(EH):
                kq = kq_pool.tile([P, SW], BF16, tag="kq", name="kq")
                nc.vector.tensor_add(
                    kq[:], kraws[eh][:],
                    q_bc[:, b * D + eh * SW: b * D + (eh + 1) * SW],
                )
                tt = tt_pool.tile([P, SW], BF16, tag="tt", name="tt")
                nc.scalar.activation(tt[:], kq[:], AF.Tanh)
                if stage < 3:
                    if eh == 0 and jg == NSQ - 1:
                        dbg = fin_pool.tile([P, NSQ], F32, tag="outt", name="dbg")
                        nc.vector.tensor_copy(dbg[:], tt[:, :NSQ])
                        nc.sync.dma_start(out[b], dbg[:])
                    continue
                # fused (tt * v) + row-sum in one native DVE op; per-eh
                # partial sums combined below.
                scr = scr_pool.tile([P, SW], BF16, tag="scr", name="scr")
                nc.vector.scalar_tensor_tensor(
                    out=scr[:],
                    in0=tt[:],
                    scalar=0.0,
                    in1=v_b[:, eh * SW:(eh + 1) * SW],
                    op0=ALU.add,
                    op1=ALU.mult,
                    accum_out=s0t[:, eh:eh + 1],
                )
            nc.vector.tensor_add(
                score_sb[:, jg:jg + 1], s0t[:, 0:1], s0t[:, 1:2]
            )

        sblk = 0
        q_emitted = False
        pending = []  # scoring chains deferred until q is available
        for b in range(NB):
            score_sb = score_pool.tile([P, NSQ], F32, tag="score", name="score_sb")
            for si, (off, w) in enumerate(strips):
                if b == 0 and si == 0:
                    mem_t = mem_t0
                else:
                    mem_f = memf_pool.tile([P, DC * SW], F32, tag="memf", name="mem_f")
                    eng = nc.sync if b == 0 else nc.scalar
                    eng.dma_start(mem_f[:, :DC * w], memS[b, :, DC * off:DC * (off + w)])
                    mem_t = memb_pool.tile([P, DC * SW], BF16, tag="memb", name="mem_t")
                    CAST.tensor_copy(mem_t[:, :DC * w], mem_f[:, :DC * w])
                for jj in range(w // P):
                    if sblk == Q_AT and not q_emitted and stage >= 2:
                        emit_q()
                        for args in pending:
                            emit_score(*args)
                        pending.clear()
                        q_emitted = True
                    jg = off // P + jj
                    kps = [
                        kps_pool.tile([P, SW], F32, tag="k", name="k_ps")
                        for _ in range(EH)
                    ]
                    for dc in range(DC):
                        stat = mem_t[:, dc * w + jj * P: dc * w + (jj + 1) * P]
                        for eh in range(EH):
                            nc.tensor.matmul(
                                kps[eh][:],
                                stat,
                                wk_b[:, dc * D + eh * SW: dc * D + (eh + 1) * SW],
                                start=(dc == 0),
                                stop=(dc == DC - 1),
                            )
                    kraws = []
                    for eh in range(EH):
                        kraw = kraw_pool.tile([P, SW], BF16, tag="kraw", name="kraw")
                        nc.scalar.copy(kraw[:], kps[eh][:])
                        kraws.append(kraw)
                    if stage < 2:
                        if jg == NSQ - 1:
                            dbg = fin_pool.tile([P, NSQ], F32, tag="outt", name="dbg")
                            nc.vector.tensor_copy(dbg[:], kraws[0][:, :NSQ])
                            nc.sync.dma_start(out[b], dbg[:])
                        sblk += 1
                        continue
                    if not q_emitted:
                        pending.append((b, jg, kraws, score_sb))
                    else:
                        emit_score(b, jg, kraws, score_sb)
                    sblk += 1
            if stage < 3:
                continue
            if stage < 25:
                outt = fin_pool.tile([P, NSQ], F32, tag="outt", name="outt")
                nc.vector.tensor_copy(outt[:], score_sb[:])
                nc.sync.dma_start(out[b], outt[:])
                continue
            # ---- masked softmax finale for batch b (compact layout) ----
            esq = fin_pool.tile([P, NSQ], F32, tag="esq", name="esq")
            nc.scalar.activation(esq[:], score_sb[:], AF.Exp)
            em = fin_pool.tile([P, NSQ], F32, tag="em", name="em")
            part = fin_pool.tile([P, 1], F32, tag="part", name="part")
            nc.vector.tensor_mul(em[:], esq[:], pad_sb[:, b * NSQ:(b + 1) * NSQ])
            nc.vector.reduce_sum(part[:], em[:], axis=mybir.AxisListType.X)
            tot_ps = sm_pool.tile([P, 1], F32, tag="small", name="tot_ps")
            nc.tensor.matmul(tot_ps[:], ones_sb[:], part[:], start=True, stop=True)
            recip = fin_pool.tile([P, 1], F32, tag="recip", name="recip")
            nc.vector.reciprocal(recip[:], tot_ps[:])
            outt = fin_pool.tile([P, NSQ], F32, tag="outt", name="outt")
            nc.vector.tensor_scalar_mul(outt[:], em[:], recip[:, 0:1])
            nc.sync.dma_start(out[b], outt[:])

    nc.compile()
    return nc


def get_program(stage=None, widths=None):
    if stage is None:
        stage = int(os.environ.get("KERNEL_STAGE", "27"))
    assert widths is not None
    key = (stage, widths)
    if key not in _CACHE:
        _CACHE[key] = _build_program(stage, widths)
    return _CACHE[key]


def prepare_in_maps(memory, target, memory_mask, Wq, Wk, v):
    memory = np.asarray(memory, dtype=np.float32)
    target = np.asarray(target, dtype=np.float32)
    Wq = np.asarray(Wq, dtype=np.float32)
    Wk = np.asarray(Wk, dtype=np.float32)
    v = np.asarray(v, dtype=np.float32)
    mask = np.asarray(memory_mask)

    # host-side sharding / layout prep (no arithmetic)
    keep_bool = ~mask                                                # [B, S]
    n_kept = keep_bool.sum(1).astype(np.int64)
    widths = make_widths(int(n_kept.max()))
    s_pad = sum(widths)
    NSQ = s_pad // P

    memT = memory.transpose(0, 2, 1)                                 # [B, D, S] view
    kept_idx = []
    kept_pad = np.empty((B, s_pad), dtype=np.int64)
    for b in range(B):
        k = np.flatnonzero(keep_bool[b])
        kept_idx.append(k)
        kept_pad[b, :len(k)] = k
        kept_pad[b, len(k):] = k[0]  # pad data: duplicate first kept column

    # memS[b, p, dc-major strip layout]: strip at offset `off`, width w
    # occupies columns [DC*off, DC*(off+w)), internally dc*w + s.
    memS = np.empty((B, P, DC * s_pad), dtype=np.float32)
    for b in range(B):
        memC = memT[b][:, kept_pad[b]]                               # [D, s_pad]
        memC4 = memC.reshape(DC, P, s_pad)
        off = 0
        for w in widths:
            blk = memC4[:, :, off:off + w]                           # [DC, P, w]
            memS[b, :, DC * off:DC * (off + w)] = (
                blk.transpose(1, 0, 2).reshape(P, DC * w))
            off += w

    # padmask in compact [P, NSQ] layout: compact index c = j*128 + p
    padm = np.zeros((B, P, NSQ), dtype=np.float32)
    for b in range(B):
        c = np.arange(s_pad)
        keepc = (c < n_kept[b]).astype(np.float32)
        padm[b] = keepc.reshape(NSQ, P).T

    wkT = np.ascontiguousarray(Wk.T)                                 # [D, D]
    wqT = np.ascontiguousarray(Wq.T)                                 # [D, D]
    tgtT = np.ascontiguousarray(target.T)                            # [D, B]
    vr = np.ascontiguousarray(np.broadcast_to(v[None, :], (P, D)))

    selm = np.zeros((NB, NB * P), dtype=np.float32)
    for b in range(NB):
        selm[b, b * P:(b + 1) * P] = 1.0

    _UNSCATTER.clear()
    _UNSCATTER["kept_idx"] = kept_idx
    _UNSCATTER["n_kept"] = n_kept
    _UNSCATTER["s_pad"] = s_pad

    in_maps = [
        {
            "memS": np.ascontiguousarray(memS[c * NB:(c + 1) * NB]),
            "wkT": wkT,
            "wqT": wqT,
            "tgtT": np.ascontiguousarray(tgtT[:, c * NB:(c + 1) * NB]),
            "vrep": vr,
            "padm": np.ascontiguousarray(padm[c * NB:(c + 1) * NB]),
            "selm": selm,
        }
        for c in range(N_CORES)
    ]
    return in_maps, widths


def unscatter_batch(out_b, batch):
    """out_b: [P, NSQ] compact normalized softmax for global batch index."""
    flat = np.asarray(out_b).T.ravel()
    full = np.zeros(S, dtype=np.float32)
    k = _UNSCATTER["kept_idx"][batch]
    full[k] = flat[:len(k)]
    return full


def gather_output(results):
    out = np.empty((B, S), dtype=np.float32)
    for c in range(N_CORES):
        o = results[c]["out"]
        for nb in range(NB):
            out[c * NB + nb] = unscatter_batch(o[nb], c * NB + nb)
    return out


def kernel(memory, target, memory_mask, Wq, Wk, v):
    from concourse.bass_utils import run_bass_kernel_spmd

    in_maps, widths = prepare_in_maps(memory, target, memory_mask, Wq, Wk, v)
    nc = get_program(widths=widths)
    res = run_bass_kernel_spmd(nc, in_maps, list(range(N_CORES)))
    return gather_output(res.results)


# revision 25
# speedup vs baseline: 1.5824x; 1.4594x over previous
"""Additive (Bahdanau) attention scoring kernel for Trainium2, 8-core SPMD.

Reference computation (B=16, S=4096, D=1024, all fp32):
    q      = target @ Wq.T                    # [B, D]
    k      = memory @ Wk.T                    # [B, S, D]
    scores = tanh(q[:, None, :] + k) @ v      # [B, S]
    out    = softmax(scores - 1e9 * mask, axis=-1)

Sharding: batch across the 8 cores (2 batches per core), weights replicated.

Host-side prep (layout + bf16 rounding): masked positions contribute exactly
0 to the reference softmax (exp(-1e9) == 0 in fp32), so memory is compacted
to the unmasked columns per batch (padded to a 128 multiple with duplicates
of the first kept column; pads are zeroed on device via padmask before the
softmax sum). Layout is strip-blocked [P, dc-major] so each strip is ONE
contiguous-per-partition DMA. Matmul operands are shipped as bf16 (validated
3.3e-3 max rel err vs the 2e-2 gate); all accumulation stays f32 on device.
The compact softmax result is unscattered to full S on the host (inverse of
the input gather).

Device design (v3 — s-on-partitions, fused q):
  - k^T tiles [s=128, e=512] = memchunk.T @ Wk chunk, accumulated over the
    8 d-chunks in PSUM (stationary = mem chunk, moving = Wk, both bf16).
    s lands on PSUM partitions.
  - q is folded into the same PSUM accumulation as a final K=2 matmul:
    sel[:, b] selects batch b's row of q_row [2, 1024] and broadcasts it
    across all 128 partitions. No DVE add, no PSUM->SBUF staging.
  - ACT: tt = tanh(psum) directly; DVE: fused (tt * v) + row-sum via
    scalar_tensor_tensor, one op per e-half, partials combined per block.
  - Softmax finale per batch on compact [128, NSQ] layout: exp (ACT),
    padmask multiply + row-sum (DVE), ones-matmul partition reduce (PE),
    reciprocal + scale (DVE). No max-shift needed: |scores| <= sum|v| ~ 8.
  - DMA: sync queue = Wq first (q matmuls lead the PE stream), then Wk and
    consts; scalar queue = all memory strips. No on-device casts.
"""

import os
from contextlib import ExitStack

import numpy as np
import ml_dtypes

import concourse.tile as tile
from concourse import bacc, mybir
import concourse.bass as bass

B, S, D = 16, 4096, 1024
N_CORES = 8
NB = B // N_CORES  # batches per core
P = 128
DC = D // P        # contraction chunks (8)
EH = 2             # e halves (2 x 512)
SW = 512           # max strip width along compacted s

F32 = mybir.dt.float32
BF16 = mybir.dt.bfloat16
AF = mybir.ActivationFunctionType
ALU = mybir.AluOpType
NPBF16 = ml_dtypes.bfloat16

_CACHE = {}
_UNSCATTER = {}


def make_widths(max_kept):
    """Strip widths covering max_kept compacted columns (128-granular).
    Two small leading strips let the PE start before the big DMAs land."""
    total = max(256, ((max_kept + 127) // 128) * 128)
    ws = []
    rem = total
    for wt in (256, 256):
        if rem >= wt + 128:
            ws.append(wt)
            rem -= wt
    while rem > SW:
        ws.append(SW)
        rem -= SW
    if rem:
        ws.append(rem)
    return tuple(ws)


def _build_program(stage, widths):
    s_pad = sum(widths)
    NSQ = s_pad // P

    nc = bacc.Bacc("TRN2", target_bir_lowering=False, debug=False)

    memS = nc.dram_tensor("memS", [NB, P, DC * s_pad], BF16, kind="ExternalInput").ap()
    wkT = nc.dram_tensor("wkT", [D, D], BF16, kind="ExternalInput").ap()
    wqT = nc.dram_tensor("wqT", [D, D], BF16, kind="ExternalInput").ap()
    tgtT = nc.dram_tensor("tgtT", [D, NB], BF16, kind="ExternalInput").ap()
    vrep = nc.dram_tensor("vrep", [P, D], BF16, kind="ExternalInput").ap()
    padm = nc.dram_tensor("padm", [NB, P, NSQ], F32, kind="ExternalInput").ap()
    selm = nc.dram_tensor("selm", [NB, NB * P], BF16, kind="ExternalInput").ap()
    out = nc.dram_tensor("out", [NB, P, NSQ], F32, kind="ExternalOutput").ap()

    with tile.TileContext(nc) as tc, ExitStack() as ctx:
        consts = ctx.enter_context(tc.tile_pool(name="consts", bufs=1))
        memb_pool = ctx.enter_context(tc.tile_pool(name="memb", bufs=4))
        tt_pool = ctx.enter_context(tc.tile_pool(name="tt", bufs=6))
        scr_pool = ctx.enter_context(tc.tile_pool(name="scr", bufs=2))
        s0_pool = ctx.enter_context(tc.tile_pool(name="s0", bufs=4))
        score_pool = ctx.enter_context(tc.tile_pool(name="score", bufs=2))
        fin_pool = ctx.enter_context(tc.tile_pool(name="fin", bufs=2))
        kps_pool = ctx.enter_context(tc.tile_pool(name="kps", bufs=6, space="PSUM"))
        sm_pool = ctx.enter_context(tc.tile_pool(name="smps", bufs=2, space="PSUM"))

        strips = []
        off = 0
        for w in widths:
            strips.append((off, w))
            off += w

        # ---- sync queue: tgt + Wq first (the q matmuls lead the PE
        # program), then Wk, then the small consts.
        tgt_b = consts.tile([P, DC * NB], BF16)
        for dc in range(DC):
            nc.sync.dma_start(tgt_b[:, dc * NB:(dc + 1) * NB], tgtT[dc * P:(dc + 1) * P, :])
        wq_b = consts.tile([P, DC * D], BF16)
        for dc in range(DC):
            nc.sync.dma_start(wq_b[:, dc * D:(dc + 1) * D], wqT[dc * P:(dc + 1) * P, :])
        sel = consts.tile([NB, NB * P], BF16)
        nc.sync.dma_start(sel[:], selm[:, :])
        wk_b = consts.tile([P, DC * D], BF16)
        for dc in range(DC):
            nc.sync.dma_start(wk_b[:, dc * D:(dc + 1) * D], wkT[dc * P:(dc + 1) * P, :])
        v_b = consts.tile([P, D], BF16)
        nc.sync.dma_start(v_b[:], vrep[:, :])
        pad_sb = consts.tile([P, NB * NSQ], F32)
        for b in range(NB):
            nc.sync.dma_start(pad_sb[:, b * NSQ:(b + 1) * NSQ], padm[b])
        ones_sb = consts.tile([P, P], F32)
        nc.vector.memset(ones_sb[:], 1.0)

        # ---- q[b, e] = sum_d target[b, d] * Wq[e, d], row layout [2, 1024]
        q_row = consts.tile([NB, D], BF16)
        for j in range(EH):
            q_ps = sm_pool.tile([NB, SW], F32, tag="small", name="q_ps")
            for dc in range(DC):
                nc.tensor.matmul(
                    q_ps[:],
                    tgt_b[:, dc * NB:(dc + 1) * NB],
                    wq_b[:, dc * D + j * SW: dc * D + (j + 1) * SW],
                    start=(dc == 0),
                    stop=(dc == DC - 1),
                )
            nc.vector.tensor_copy(q_row[:, j * SW:(j + 1) * SW], q_ps[:])

        def emit_score(b, jg, kps, score_sb):
            s0t = s0_pool.tile([P, EH], F32, tag="s0", name="s0t")
            for eh in range(EH):
                tt = tt_pool.tile([P, SW], BF16, tag="tt", name="tt")
                nc.scalar.activation(tt[:], kps[eh][:], AF.Tanh)
                if stage < 3:
                    if eh == 0 and jg == NSQ - 1:
                        dbg = fin_pool.tile([P, NSQ], F32, tag="outt", name="dbg")
                        nc.vector.tensor_copy(dbg[:], tt[:, :NSQ])
                        nc.sync.dma_start(out[b], dbg[:])
                    continue
                # fused (tt * v) + row-sum in one native DVE op
                scr = scr_pool.tile([P, SW], BF16, tag="scr", name="scr")
                nc.vector.scalar_tensor_tensor(
                    out=scr[:],
                    in0=tt[:],
                    scalar=0.0,
                    in1=v_b[:, eh * SW:(eh + 1) * SW],
                    op0=ALU.add,
                    op1=ALU.mult,
                    accum_out=s0t[:, eh:eh + 1],
                )
            if stage >= 3:
                nc.vector.tensor_add(
                    score_sb[:, jg:jg + 1], s0t[:, 0:1], s0t[:, 1:2]
                )

        for b in range(NB):
            score_sb = score_pool.tile([P, NSQ], F32, tag="score", name="score_sb")
            for si, (off, w) in enumerate(strips):
                mem_t = memb_pool.tile([P, DC * SW], BF16, tag="memb", name="mem_t")
                nc.scalar.dma_start(mem_t[:, :DC * w], memS[b, :, DC * off:DC * (off + w)])
                for jj in range(w // P):
                    jg = off // P + jj
                    kps = [
                        kps_pool.tile([P, SW], F32, tag="k", name="k_ps")
                        for _ in range(EH)
                    ]
                    for dc in range(DC):
                        stat = mem_t[:, dc * w + jj * P: dc * w + (jj + 1) * P]
                        for eh in range(EH):
                            nc.tensor.matmul(
                                kps[eh][:],
                                stat,
                                wk_b[:, dc * D + eh * SW: dc * D + (eh + 1) * SW],
                                start=(dc == 0),
                                stop=False,
                            )
                    # fold q into the accumulation: psum[s, e] += q[b, e]
                    for eh in range(EH):
                        nc.tensor.matmul(
                            kps[eh][:],
                            sel[:, b * P:(b + 1) * P],
                            q_row[:, eh * SW:(eh + 1) * SW],
                            start=False,
                            stop=True,
                        )
                    emit_score(b, jg, kps, score_sb)
            if stage < 3:
                continue
            if stage < 25:
                outt = fin_pool.tile([P, NSQ], F32, tag="outt", name="outt")
                nc.vector.tensor_copy(outt[:], score_sb[:])
                nc.sync.dma_start(out[b], outt[:])
                continue
            # ---- masked softmax finale for batch b (compact layout) ----
            esq = fin_pool.tile([P, NSQ], F32, tag="esq", name="esq")
            nc.scalar.activation(esq[:], score_sb[:], AF.Exp)
            em = fin_pool.tile([P, NSQ], F32, tag="em", name="em")
            part = fin_pool.tile([P, 1], F32, tag="part", name="part")
            nc.vector.tensor_mul(em[:], esq[:], pad_sb[:, b * NSQ:(b + 1) * NSQ])
            nc.vector.reduce_sum(part[:], em[:], axis=mybir.AxisListType.X)
            tot_ps = sm_pool.tile([P, 1], F32, tag="small", name="tot_ps")
            nc.tensor.matmul(tot_ps[:], ones_sb[:], part[:], start=True, stop=True)
            recip = fin_pool.tile([P, 1], F32, tag="recip", name="recip")
            nc.vector.reciprocal(recip[:], tot_ps[:])
            outt = fin_pool.tile([P, NSQ], F32, tag="outt", name="outt")
            nc.vector.tensor_scalar_mul(outt[:], em[:], recip[:, 0:1])
            nc.sync.dma_start(out[b], outt[:])

    nc.compile()
    return nc


def get_program(stage=None, widths=None):
    if stage is None:
        stage = int(os.environ.get("KERNEL_STAGE", "27"))
    assert widths is not None
    key = (stage, widths)
    if key not in _CACHE:
        _CACHE[key] = _build_program(stage, widths)
    return _CACHE[key]


def prepare_in_maps(memory, target, memory_mask, Wq, Wk, v):
    memory = np.asarray(memory, dtype=np.float32)
    target = np.asarray(target, dtype=np.float32)
    Wq = np.asarray(Wq, dtype=np.float32)
    Wk = np.asarray(Wk, dtype=np.float32)
    v = np.asarray(v, dtype=np.float32)
    mask = np.asarray(memory_mask)

    # host-side sharding / layout prep
    keep_bool = ~mask                                                # [B, S]
    n_kept = keep_bool.sum(1).astype(np.int64)
    widths = make_widths(int(n_kept.max()))
    s_pad = sum(widths)
    NSQ = s_pad // P

    memT = memory.transpose(0, 2, 1)                                 # [B, D, S] view
    kept_idx = []
    kept_pad = np.empty((B, s_pad), dtype=np.int64)
    for b in range(B):
        k = np.flatnonzero(keep_bool[b])
        kept_idx.append(k)
        kept_pad[b, :len(k)] = k
        kept_pad[b, len(k):] = k[0]  # pad data: duplicate first kept column

    # memS[b, p, dc-major strip layout]: strip at offset `off`, width w
    # occupies columns [DC*off, DC*(off+w)), internally dc*w + s.
    memS = np.empty((B, P, DC * s_pad), dtype=NPBF16)
    for b in range(B):
        memC = memT[b][:, kept_pad[b]].astype(NPBF16)                # [D, s_pad]
        memC4 = memC.reshape(DC, P, s_pad)
        off = 0
        for w in widths:
            blk = memC4[:, :, off:off + w]                           # [DC, P, w]
            memS[b, :, DC * off:DC * (off + w)] = (
                blk.transpose(1, 0, 2).reshape(P, DC * w))
            off += w

    # padmask in compact [P, NSQ] layout: compact index c = j*128 + p
    padm = np.zeros((B, P, NSQ), dtype=np.float32)
    for b in range(B):
        c = np.arange(s_pad)
        keepc = (c < n_kept[b]).astype(np.float32)
        padm[b] = keepc.reshape(NSQ, P).T

    wkT = np.ascontiguousarray(Wk.T).astype(NPBF16)                  # [D, D]
    wqT = np.ascontiguousarray(Wq.T).astype(NPBF16)                  # [D, D]
    tgtT = np.ascontiguousarray(target.T).astype(NPBF16)             # [D, B]
    vr = np.ascontiguousarray(np.broadcast_to(v[None, :], (P, D))).astype(NPBF16)

    selm = np.zeros((NB, NB * P), dtype=NPBF16)
    for b in range(NB):
        selm[b, b * P:(b + 1) * P] = 1.0

    _UNSCATTER.clear()
    _UNSCATTER["kept_idx"] = kept_idx
    _UNSCATTER["n_kept"] = n_kept
    _UNSCATTER["s_pad"] = s_pad

    in_maps = [
        {
            "memS": np.ascontiguousarray(memS[c * NB:(c + 1) * NB]),
            "wkT": wkT,
            "wqT": wqT,
            "tgtT": np.ascontiguousarray(tgtT[:, c * NB:(c + 1) * NB]),
            "vrep": vr,
            "padm": np.ascontiguousarray(padm[c * NB:(c + 1) * NB]),
            "selm": selm,
        }
        for c in range(N_CORES)
    ]
    return in_maps, widths


def unscatter_batch(out_b, batch):
    """out_b: [P, NSQ] compact normalized softmax for global batch index."""
    flat = np.asarray(out_b).T.ravel()
    full = np.zeros(S, dtype=np.float32)
    k = _UNSCATTER["kept_idx"][batch]
    full[k] = flat[:len(k)]
    return full


def gather_output(results):
    out = np.empty((B, S), dtype=np.float32)
    for c in range(N_CORES):
        o = results[c]["out"]
        for nb in range(NB):
            out[c * NB + nb] = unscatter_batch(o[nb], c * NB + nb)
    return out


def kernel(memory, target, memory_mask, Wq, Wk, v):
    from concourse.bass_utils import run_bass_kernel_spmd

    in_maps, widths = prepare_in_maps(memory, target, memory_mask, Wq, Wk, v)
    nc = get_program(widths=widths)
    res = run_bass_kernel_spmd(nc, in_maps, list(range(N_CORES)))
    return gather_output(res.results)
